# revision 32
# baseline (speedup 1.0000x reference)
"""DecoderRNN Trainium2 kernel (8 NeuronCores).

Sharding: batch-parallel recurrence (16 rows/core), vocab-parallel output
projection (4000 cols/core). Hidden-state history exchanged via chunked
AllGather collectives overlapped with the recurrence.

Self-contained: hardcodes all shapes from the problem spec.
"""
import math
from contextlib import ExitStack

import numpy as np
import ml_dtypes

import concourse.bacc as bacc
import concourse.bass as bass
import concourse.tile as tile
from concourse import mybir
from concourse.bass import AP
from concourse.masks import make_identity

F32 = mybir.dt.float32
BF16 = mybir.dt.bfloat16
I32 = mybir.dt.int32
AF = mybir.ActivationFunctionType

# problem constants
B, L, H, V, WORD, T = 128, 64, 512, 32000, 512, 32
NC = 8            # cores
BL = B // NC      # local batch rows = 16
NR = BL * L       # local attention rows = 1024
RK = NR // 128    # row chunks = 8
HK = H // 128     # h chunks = 4
TS = T - 1        # decode steps = 31
VL = V // NC      # local vocab = 4000
G3 = 3 * H        # 1536

# phase-2 grouping (sumexp allreduce granularity) and AG chunking
P2G = 4           # t's per phase-2 group
AGC = 8           # t's per AllGather chunk


def _mm(nc, out, lhsT, rhs, start, stop):
    nc.tensor.matmul(out, lhsT, rhs, start=start, stop=stop)


def build_program(t_steps=TS, n_cores=NC):
    """Builds the SPMD Bass program. Returns compiled nc."""
    nc = bacc.Bacc("TRN2", target_bir_lowering=False, debug=False,
                   num_devices=n_cores)
    rg = [list(range(n_cores))]
    n_ag = math.ceil(t_steps / AGC)
    n_g = math.ceil(t_steps / P2G)

    # ---- I/O tensors (per-core data via in_maps) ----
    def din(name, shape, dt=F32):
        return nc.dram_tensor(name, shape, dt, kind="ExternalInput")

    enc_nat = din("enc_nat", [RK, 128, H])          # rows (b*64+l)
    encT = din("encT", [HK, 128, NR])
    hid0 = din("hid0", [BL, H])
    hidT0 = din("hidT0", [HK, 128, BL], BF16)
    tgt_idx = din("tgt_idx", [4, 128, 1], I32)      # rows t*16+b, padded 512
    embW = din("embW", [V, WORD])
    w1eT = din("w1eT", [HK, 128, H])
    w1hT = din("w1hT", [HK, 128, H], BF16)
    w2T = din("w2T", [HK, 128, H], BF16)
    w3T = din("w3T", [HK, 128, H], BF16)
    vT = din("vT", [HK, 128, 1], BF16)
    b1 = din("b1", [128, HK])
    b2 = din("b2", [128, HK])
    b3 = din("b3", [128, HK])
    wiheT = din("wiheT", [HK, 128, G3])
    wihcT = din("wihcT", [HK, 128, G3], BF16)
    whhT = din("whhT", [HK, 128, G3], BF16)
    bih = din("bih", [1, G3])
    bhh = din("bhh", [1, G3])
    outWT = din("outWT", [HK, 128, VL], BF16)
    outb = din("outb", [1, VL], BF16)
    out_lp = nc.dram_tensor("out_lp", [B, t_steps, VL], F32, kind="ExternalOutput")

    with tile.TileContext(nc) as tc, ExitStack() as top:
        dram = top.enter_context(tc.tile_pool(name="dram", bufs=1, space="DRAM"))
        hist = dram.tile([t_steps, HK, 128, BL], BF16)
        gats = [dram.tile([n_cores, min(AGC, t_steps - g * AGC), HK, 128, BL], BF16,
                          name=f"gat{g}") for g in range(n_ag)]
        ar_ins = [dram.tile([128, min(P2G, t_steps - g * P2G)], F32, name=f"ari{g}")
                  for g in range(n_g)]
        ar_outs = [dram.tile([128, min(P2G, t_steps - g * P2G)], F32, name=f"aro{g}")
                   for g in range(n_g)]

        # ---------------- persistent SBUF (whole kernel) ----------------
        per = top.enter_context(tc.tile_pool(name="per", bufs=1))
        ident = per.tile([128, 128], F32)
        make_identity(nc, ident[:])
        ones32 = per.tile([1, 128], F32)
        nc.gpsimd.memset(ones32[:], 1.0)
        onesb = per.tile([1, 128], BF16)
        nc.gpsimd.memset(onesb[:], 1.0)
        sumexp = per.tile([128, t_steps], F32)

        with ExitStack() as ph1:
            p1 = ph1.enter_context(tc.tile_pool(name="p1", bufs=1))
            # persistent phase-1 tensors
            enc_sb = p1.tile([128, RK, H + 1], F32)
            nc.sync.dma_start(enc_sb[:, :, 0:H], enc_nat.ap().rearrange("k p h -> p k h"))
            nc.gpsimd.memset(enc_sb[:, :, H:H + 1], 1.0)
            w1hT_sb = p1.tile([128, HK, H], BF16)
            nc.sync.dma_start(w1hT_sb[:], w1hT.ap().rearrange("k p h -> p k h"))
            w2T_sb = p1.tile([128, HK, H], BF16)
            nc.sync.dma_start(w2T_sb[:], w2T.ap().rearrange("k p h -> p k h"))
            w3T_sb = p1.tile([128, HK, H], BF16)
            nc.sync.dma_start(w3T_sb[:], w3T.ap().rearrange("k p h -> p k h"))
            vT_sb = p1.tile([128, HK], BF16)
            nc.sync.dma_start(vT_sb[:], vT.ap().rearrange("k p one -> p (k one)"))
            b1_sb = p1.tile([128, HK], F32)
            nc.sync.dma_start(b1_sb[:], b1.ap())
            b2_sb = p1.tile([128, HK], F32)
            nc.sync.dma_start(b2_sb[:], b2.ap())
            b3_sb = p1.tile([128, HK], F32)
            nc.sync.dma_start(b3_sb[:], b3.ap())
            wihcT_sb = p1.tile([128, HK, G3], BF16)
            nc.sync.dma_start(wihcT_sb[:], wihcT.ap().rearrange("k p h -> p k h"))
            whhT_sb = p1.tile([128, HK, G3], BF16)
            nc.sync.dma_start(whhT_sb[:], whhT.ap().rearrange("k p h -> p k h"))
            bhh_sb = p1.tile([1, G3], F32)
            nc.sync.dma_start(bhh_sb[:], bhh.ap())
            encprojT = p1.tile([128, HK, BL, L], BF16)
            gi_emb = p1.tile([128, 4, G3], BF16)
            mask_sb = p1.tile([128, RK, BL], F32)
            nc.gpsimd.memset(mask_sb[:], 0.0)

            # pools for per-step working tiles
            hidp = ph1.enter_context(tc.tile_pool(name="hidp", bufs=2))
            wka = ph1.enter_context(tc.tile_pool(name="wka", bufs=1))
            wk = ph1.enter_context(tc.tile_pool(name="wk", bufs=2))
            gw = ph1.enter_context(tc.tile_pool(name="gw", bufs=1))
            # PSUM budget is 8 banks total, statically reserved per pool:
            # pd 3 (dense m-tiles) + pgg 3 (gh/gi/phase0) + pmisc 2 = 8
            pd = ph1.enter_context(tc.tile_pool(name="pd", bufs=3, space="PSUM"))
            pgg = ph1.enter_context(tc.tile_pool(name="pgg", bufs=1, space="PSUM"))
            pmisc = ph1.enter_context(tc.tile_pool(name="pmisc", bufs=1, space="PSUM"))

            # ---------------- phase 0: one-time precompute ----------------
            with ExitStack() as ph0:
                p0 = ph0.enter_context(tc.tile_pool(name="p0", bufs=1))
                p0s = ph0.enter_context(tc.tile_pool(name="p0s", bufs=2))
                w1eT_sb = p0.tile([128, HK, H], F32)
                nc.sync.dma_start(w1eT_sb[:], w1eT.ap().rearrange("k p h -> p k h"))
                bih_sb = p0.tile([1, G3], F32)
                nc.sync.dma_start(bih_sb[:], bih.ap())
                embT = p0.tile([128, HK, 4, 128], F32)
                with ExitStack() as ph00:
                    p00 = ph00.enter_context(tc.tile_pool(name="p00", bufs=1))
                    idx_sb = p00.tile([128, 4], I32)
                    nc.sync.dma_start(idx_sb[:], tgt_idx.ap().rearrange("r p one -> p (r one)"))
                    embg = p00.tile([128, 4, WORD], F32)
                    for r in range(4):
                        nc.gpsimd.indirect_dma_start(
                            out=embg[:, r, :], out_offset=None, in_=embW.ap(),
                            in_offset=bass.IndirectOffsetOnAxis(ap=idx_sb[:, r:r + 1], axis=0))
                    # transpose embeddings: embT[p=h%128, k, r, rows128]
                    for r in range(4):
                        for k in range(HK):
                            pt = pgg.tile([128, 128], F32, tag="pgg")
                            nc.tensor.transpose(pt[:], embg[:, r, k * 128:(k + 1) * 128],
                                                ident[:])
                            nc.vector.tensor_copy(embT[:, k, r, :], pt[:])
                # gi_emb[p=row%128, r, f] = emb @ Wih_e.T + bih   (stream Wih_e chunks)
                for r in range(4):
                    pge = pgg.tile([128, G3], F32, tag="pgg")
                    for k in range(HK):
                        wch = p0s.tile([128, G3], F32, tag="wch")
                        nc.sync.dma_start(wch[:], wiheT.ap()[k])
                        for j in range(3):
                            _mm(nc, pge[:, j * 512:(j + 1) * 512], embT[:, k, r, :],
                                wch[:, j * 512:(j + 1) * 512], k == 0, False)
                    for j in range(3):
                        _mm(nc, pge[:, j * 512:(j + 1) * 512], ones32[:],
                            bih_sb[:, j * 512:(j + 1) * 512], False, True)
                    nc.vector.tensor_copy(gi_emb[:, r, :], pge[:])
                # encprojT[p=h'%128, m, b, l] = W1e @ enc.T   (stream enc.T chunks)
                for m in range(HK):
                    pep = pgg.tile([128, NR], F32, tag="pgg")
                    for k in range(HK):
                        ech = p0s.tile([128, NR], F32, tag="ech")
                        nc.sync.dma_start(ech[:], encT.ap()[k])
                        for j in range(2):
                            _mm(nc, pep[:, j * 512:(j + 1) * 512],
                                w1eT_sb[:, k, m * 128:(m + 1) * 128],
                                ech[:, j * 512:(j + 1) * 512], k == 0, k == HK - 1)
                    nc.vector.tensor_copy(
                        encprojT[:, m, :, :], pep[:].rearrange("p (b l) -> p b l", b=BL))

            # ---------------- phase 1: recurrence ----------------
            hid = hidp.tile([BL, H], F32, tag="hid")
            nc.sync.dma_start(hid[:], hid0.ap())
            hidT = hidp.tile([128, HK, BL], BF16, tag="hidT")
            nc.sync.dma_start(hidT[:], hidT0.ap().rearrange("k p b -> p k b"))

            for t in range(t_steps):
                # gh = Whh @ hid + bhh -> evacuated to SBUF (psum slot shared w/ gi)
                pgh = pgg.tile([BL, G3], F32, tag="pgg")
                for k in range(HK):
                    for j in range(3):
                        _mm(nc, pgh[:, j * 512:(j + 1) * 512], hidT[:, k, :],
                            whhT_sb[:, k, j * 512:(j + 1) * 512], k == 0, False)
                for j in range(3):
                    _mm(nc, pgh[:, j * 512:(j + 1) * 512], ones32[:, 0:BL],
                        bhh_sb[:, j * 512:(j + 1) * 512], False, True)
                gh_sb = gw.tile([BL, G3], F32, tag="gh_sb")
                nc.vector.tensor_copy(gh_sb[:], pgh[:])

                # hidproj = W1h @ hid
                php = pmisc.tile([128, HK, BL], F32, tag="pmisc")
                for m in range(HK):
                    for k in range(HK):
                        _mm(nc, php[:, m, :], w1hT_sb[:, k, m * 128:(m + 1) * 128],
                            hidT[:, k, :], k == 0, k == HK - 1)

                # a1 = tanh(encproj + hidproj + b1)  [h-part layout]
                a1T = wka.tile([128, HK, NR], BF16, tag="a1T")
                for m in range(HK):
                    pre = wk.tile([128, BL, L], F32, tag="a1pre")
                    hb = php[:, m, :]
                    hb = AP(tensor=hb.tensor, offset=hb.offset, ap=hb.ap + [[0, L]])
                    nc.vector.tensor_add(pre[:], encprojT[:, m, :, :], hb)
                    nc.scalar.activation(
                        out=a1T[:, m, :].rearrange("p (b l) -> p b l", b=BL), in_=pre[:],
                        func=AF.Tanh, bias=b1_sb[:, m:m + 1], scale=1.0)

                # dense2 / dense3 with tanh, half-split for psum
                # a3T reuses a1T's slot (a1 dead once dense2 is done)
                a2T = wka.tile([128, HK, NR], BF16, tag="a2T")
                a3T = wka.tile([128, HK, NR], BF16, tag="a1T")
                for (src, dst, wT, bias) in ((a1T, a2T, w2T_sb, b2_sb),
                                             (a2T, a3T, w3T_sb, b3_sb)):
                    for hf in range(2):
                        sl = slice(hf * 512, (hf + 1) * 512)
                        for m in range(HK):
                            pdt = pd.tile([128, 512], F32, tag="pd")
                            for k in range(HK):
                                _mm(nc, pdt[:], wT[:, k, m * 128:(m + 1) * 128],
                                    src[:, k, sl], k == 0, k == HK - 1)
                            nc.scalar.activation(out=dst[:, m, sl], in_=pdt[:],
                                                 func=AF.Tanh, bias=bias[:, m:m + 1],
                                                 scale=1.0)

                # eT[p=row%128, m] = a3 . v ; exp
                pe = pmisc.tile([128, RK], F32, tag="pmisc")
                for m in range(RK):
                    for k in range(HK):
                        _mm(nc, pe[:, m:m + 1], a3T[:, k, m * 128:(m + 1) * 128],
                            vT_sb[:, k:k + 1], k == 0, k == HK - 1)
                expeT = gw.tile([128, RK], F32, tag="expeT")
                nc.scalar.activation(out=expeT[:], in_=pe[:], func=AF.Exp)

                # mask strips (zeros persist from phase 0)
                for k in range(RK):
                    nc.vector.tensor_copy(mask_sb[0:64, k, 2 * k:2 * k + 1],
                                          expeT[0:64, k:k + 1])
                    nc.vector.tensor_copy(mask_sb[64:128, k, 2 * k + 1:2 * k + 2],
                                          expeT[64:128, k:k + 1])

                # ctxu[b, h] (+ Z in col H) = mask.T @ [enc | 1]
                pcu = pmisc.tile([BL, H + 1], F32, tag="pmisc")
                for k in range(RK):
                    _mm(nc, pcu[:, 0:H], mask_sb[:, k, :], enc_sb[:, k, 0:H],
                        k == 0, k == RK - 1)
                    _mm(nc, pcu[:, H:H + 1], mask_sb[:, k, :], enc_sb[:, k, H:H + 1],
                        k == 0, k == RK - 1)
                rcpZ = gw.tile([BL, 1], F32, tag="rcpZ")
                nc.vector.reciprocal(rcpZ[:], pcu[:, H:H + 1])
                ctxu = gw.tile([BL, H], F32, tag="ctxu")
                nc.vector.tensor_copy(ctxu[:], pcu[:, 0:H])
                diag = gw.tile([BL, BL], F32, tag="diag")
                nc.vector.tensor_scalar_mul(diag[:], ident[0:BL, 0:BL], rcpZ[:])

                # ctxT[h, b] = ctxu.T scaled by rcpZ  (transpose+scale via diag mm)
                pct = pmisc.tile([128, HK, BL], F32, tag="pmisc")
                for m in range(HK):
                    _mm(nc, pct[:, m, :], ctxu[:, m * 128:(m + 1) * 128], diag[:],
                        True, True)
                ctxT = gw.tile([128, HK, BL], BF16, tag="ctxT")
                nc.vector.tensor_copy(ctxT[:], pct[:])

                # gi_ctx = Wih_c @ ctx
                pgi = pgg.tile([BL, G3], F32, tag="pgg")
                for k in range(HK):
                    for j in range(3):
                        _mm(nc, pgi[:, j * 512:(j + 1) * 512], ctxT[:, k, :],
                            wihcT_sb[:, k, j * 512:(j + 1) * 512], k == 0, k == HK - 1)

                # gates (stage this step's gi_emb rows to partitions 0:16 via DMA;
                # engine ops need 32-aligned start partitions)
                po = (t % 8) * BL
                tc_ = t // 8
                ge_t = wk.tile([BL, G3], BF16, tag="ge_t")
                nc.sync.dma_start(ge_t[:], gi_emb[po:po + BL, tc_, :])
                ge_rz = ge_t[:, 0:2 * H]
                ge_n = ge_t[:, 2 * H:G3]
                rz = gw.tile([BL, 2 * H], F32, tag="rz")
                nc.vector.tensor_add(rz[:], pgi[:, 0:2 * H], gh_sb[:, 0:2 * H])
                nc.vector.tensor_add(rz[:], rz[:], ge_rz)
                nc.scalar.activation(out=rz[:], in_=rz[:], func=AF.Sigmoid)
                n1 = gw.tile([BL, H], F32, tag="n1")
                nc.vector.tensor_add(n1[:], pgi[:, 2 * H:G3], ge_n)
                n2 = gw.tile([BL, H], F32, tag="n2")
                nc.vector.tensor_mul(n2[:], rz[:, 0:H], gh_sb[:, 2 * H:G3])
                nc.vector.tensor_add(n1[:], n1[:], n2[:])
                nc.scalar.activation(out=n1[:], in_=n1[:], func=AF.Tanh)
                nc.vector.tensor_sub(n2[:], hid[:], n1[:])         # d = hid - n
                nc.vector.tensor_mul(n2[:], rz[:, H:2 * H], n2[:])  # z*d
                hid = hidp.tile([BL, H], F32, tag="hid")
                nc.vector.tensor_add(hid[:], n1[:], n2[:])

                # hidT (f32) + bf16 copy for history
                pht = pmisc.tile([128, HK, BL], F32, tag="pmisc")
                for k in range(HK):
                    nc.tensor.transpose(pht[:, k, :], hid[:, k * 128:(k + 1) * 128],
                                        ident[0:BL, 0:BL])
                hidT = hidp.tile([128, HK, BL], BF16, tag="hidT")
                nc.vector.tensor_copy(hidT[:], pht[:])
                nc.sync.dma_start(hist[t].rearrange("k p b -> p k b"), hidT[:])

                # chunked AllGather of history
                if (t + 1) % AGC == 0 or t == t_steps - 1:
                    g = t // AGC
                    nc.gpsimd.collective_compute(
                        "AllGather", mybir.AluOpType.bypass, replica_groups=rg,
                        ins=[hist[g * AGC:g * AGC + gats[g].shape[1]].opt()],
                        outs=[gats[g][:].opt()])

        # ---------------- phase 2: output projection + log-softmax ----------------
        with ExitStack() as ph2:
            p2 = ph2.enter_context(tc.tile_pool(name="p2", bufs=1))
            outWT_sb = p2.tile([128, HK, VL], BF16)
            nc.sync.dma_start(outWT_sb[:], outWT.ap().rearrange("k p v -> p k v"))
            outb_sb = p2.tile([1, VL], BF16)
            nc.sync.dma_start(outb_sb[:], outb.ap())
            stp = ph2.enter_context(tc.tile_pool(name="stp", bufs=2))
            w2p = ph2.enter_context(tc.tile_pool(name="w2p", bufs=3))
            pl = ph2.enter_context(tc.tile_pool(name="pl", bufs=2, space="PSUM"))

            nchunks = [(i * 512, min((i + 1) * 512, VL)) for i in range((VL + 511) // 512)]
            for g in range(n_g):
                g0 = g * P2G
                gsz = min(P2G, t_steps - g0)
                stash = stp.tile([128, gsz, VL], BF16, tag="stash")
                for tt in range(gsz):
                    t = g0 + tt
                    htf = w2p.tile([128, HK, n_cores, BL], BF16, tag="htf")
                    for k in range(HK):
                        nc.sync.dma_start(
                            htf[:, k, :, :], gats[t // AGC][:, t % AGC, k, :, :]
                            .rearrange("c p b -> p c b"))
                    for hf in range(2):
                        cs = nchunks[hf * 4:(hf + 1) * 4]
                        plg = pl.tile([128, 2048], F32, tag="plg")
                        for k in range(HK):
                            for (c0, c1) in cs:
                                _mm(nc, plg[:, c0 - cs[0][0]:c1 - cs[0][0]],
                                    htf[:, k, :, :], outWT_sb[:, k, c0:c1], k == 0, False)
                        for (c0, c1) in cs:
                            _mm(nc, plg[:, c0 - cs[0][0]:c1 - cs[0][0]], onesb[:],
                                outb_sb[:, c0:c1], False, True)
                        w = cs[-1][1] - cs[0][0]
                        exps = w2p.tile([128, 2048], BF16, tag="exps")
                        se = w2p.tile([128, 1], F32, tag=f"se{hf}")
                        nc.scalar.activation(out=exps[:, 0:w], in_=plg[:, 0:w],
                                             func=AF.Exp, accum_out=se[:])
                        nc.scalar.activation(
                            out=stash[:, tt, cs[0][0]:cs[-1][1]], in_=plg[:, 0:w],
                            func=AF.Copy)
                        if hf == 0:
                            se0 = se
                        else:
                            nc.vector.tensor_add(sumexp[:, t:t + 1], se0[:], se[:])
                # exchange sumexp partials for this group
                nc.sync.dma_start(ar_ins[g][:], sumexp[:, g0:g0 + gsz])
                nc.gpsimd.collective_compute(
                    "AllReduce", mybir.AluOpType.add, replica_groups=rg,
                    ins=[ar_ins[g][:].opt()], outs=[ar_outs[g][:].opt()])
                gse = w2p.tile([128, gsz], F32, tag="gse")
                nc.sync.dma_start(gse[:], ar_outs[g][:])
                lnz = w2p.tile([128, gsz], F32, tag="lnz")
                nc.scalar.activation(out=lnz[:], in_=gse[:], func=AF.Ln)
                nlz = w2p.tile([128, gsz], F32, tag="nlz")
                nc.vector.tensor_scalar_mul(nlz[:], lnz[:], -1.0)
                for tt in range(gsz):
                    t = g0 + tt
                    lp = w2p.tile([128, VL], F32, tag="lp")
                    nc.vector.tensor_scalar_add(lp[:], stash[:, tt, :], nlz[:, tt:tt + 1])
                    nc.sync.dma_start(out_lp.ap()[:, t, :], lp[:])

    nc.compile()
    return nc


_NC_CACHE = {}


def _get_program(t_steps=TS, n_cores=NC):
    key = (t_steps, n_cores)
    if key not in _NC_CACHE:
        _NC_CACHE[key] = build_program(t_steps, n_cores)
    return _NC_CACHE[key]


def make_in_maps(inputs, t_steps=TS, n_cores=NC):
    """Host-side shard/layout prep. Pure data movement + dtype casts."""
    enc = np.asarray(inputs["encoder_outputs"], np.float32)
    ehid = np.asarray(inputs["encoder_hidden"], np.float32)
    targets = np.asarray(inputs["targets"])
    embW = np.ascontiguousarray(np.asarray(inputs["embed_W"], np.float32))
    aW1 = np.asarray(inputs["att_W1"], np.float32)
    aW2 = np.asarray(inputs["att_W2"], np.float32)
    aW3 = np.asarray(inputs["att_W3"], np.float32)
    ab1 = np.asarray(inputs["att_b1"], np.float32)
    ab2 = np.asarray(inputs["att_b2"], np.float32)
    ab3 = np.asarray(inputs["att_b3"], np.float32)
    av = np.asarray(inputs["att_v"], np.float32)
    gWih = np.asarray(inputs["gru_Wih"], np.float32)
    gWhh = np.asarray(inputs["gru_Whh"], np.float32)
    gbih = np.asarray(inputs["gru_bih"], np.float32)
    gbhh = np.asarray(inputs["gru_bhh"], np.float32)
    oW = np.asarray(inputs["out_W"], np.float32)
    ob = np.asarray(inputs["out_b"], np.float32)

    def chunkT(w, dt=np.float32):  # (out,in)->(in,out) h-chunked: (HK,128,out)
        wt = np.ascontiguousarray(w.T.astype(dt))
        return wt.reshape(HK, 128, w.shape[0])

    shared = {
        "embW": embW,
        "w1eT": chunkT(aW1[:, :H]),
        "w1hT": chunkT(aW1[:, H:], ml_dtypes.bfloat16),
        "w2T": chunkT(aW2, ml_dtypes.bfloat16), "w3T": chunkT(aW3, ml_dtypes.bfloat16),
        "vT": np.ascontiguousarray(av[0].astype(ml_dtypes.bfloat16)).reshape(HK, 128, 1),
        "b1": np.ascontiguousarray(ab1.reshape(HK, 128).T),
        "b2": np.ascontiguousarray(ab2.reshape(HK, 128).T),
        "b3": np.ascontiguousarray(ab3.reshape(HK, 128).T),
        "wiheT": chunkT(gWih[:, :WORD]),
        "wihcT": chunkT(gWih[:, WORD:], ml_dtypes.bfloat16),
        "whhT": chunkT(gWhh, ml_dtypes.bfloat16),
        "bih": gbih.reshape(1, G3).astype(np.float32),
        "bhh": gbhh.reshape(1, G3).astype(np.float32),
    }
    in_maps = []
    for c in range(n_cores):
        bl0 = c * BL
        enc_l = enc[bl0:bl0 + BL].reshape(NR, H)
        idx = np.zeros(512, np.int32)
        idx[: BL * t_steps] = targets[bl0:bl0 + BL, :t_steps].T.astype(np.int32).ravel()
        m = dict(shared)
        m["enc_nat"] = np.ascontiguousarray(enc_l.reshape(RK, 128, H))
        m["encT"] = np.ascontiguousarray(enc_l.T).reshape(HK, 128, NR)
        m["hid0"] = np.ascontiguousarray(ehid[0, bl0:bl0 + BL])
        m["hidT0"] = np.ascontiguousarray(
            ehid[0, bl0:bl0 + BL].T.astype(ml_dtypes.bfloat16)).reshape(HK, 128, BL)
        m["tgt_idx"] = idx.reshape(4, 128, 1)
        m["outWT"] = np.ascontiguousarray(oW[c * VL:(c + 1) * VL].T.astype(ml_dtypes.bfloat16)).reshape(HK, 128, VL)
        m["outb"] = ob[c * VL:(c + 1) * VL].reshape(1, VL).astype(ml_dtypes.bfloat16)
        in_maps.append(m)
    return in_maps


def run(inputs, trace=False, **trace_kw):
    from concourse import bass_utils
    nc = _get_program()
    in_maps = make_in_maps(inputs)
    res = bass_utils.run_bass_kernel_spmd(nc, in_maps, core_ids=list(range(NC)),
                                          trace=trace, **trace_kw)
    out = np.concatenate([res.results[c]["out_lp"] for c in range(NC)], axis=2)
    return out, res


def kernel(**inputs):
    return run(inputs)[0]
